# revision 2
# baseline (speedup 1.0000x reference)
"""Trainium2 Bass kernel for per-batch spatial self-attention — fp8 DoubleRow.

Math (per core, one batch image, channels-major x:[256,4096]):
  scores[n,m] = x[:,n]^T A x[:,m] + g[m],   A = Wq^T Wk,  g = x^T (Wk^T bq)
  (k-bias dropped: softmax-invariant; q-bias enters only via g.)
  P = exp(scores/16);  out[o,n] = (V'^T P)[o,n] / den[n] where
  V'[m,o] = (x[:,m]^T Wv^T + bv[o])*(1+g[m]/16)   (e^{g/16} linearized)
  den[n]  = sum_m P[m,n]*(1+g[m]/16)

Precision plan:
  - Projections (Kb = 64*A x, V', g) run in f32r from unquantized x.
  - Score/attn matmuls run fp8e4 DoubleRow (contraction 256 in one MM).
  - The q-side x is quantized at 4 slightly different scales lam_i
    (quarter-octave apart); key-chunk c uses version c%4 and the exp scale
    divides lam back out.  This decorrelates the per-query quantization
    error field across chunks (the dominant fp8 error term).
  - den picks up the g correction via a second fp8 lhsT column (g/16) in
    the rowsum matmul; the two accumulator rows are folded by a tiny
    select-matmul before the reciprocal.

Device layout:
  x8v[k] [128,2,4096] fp8   x8v[k][p,j,n]  = q8(lam_k * x[p+128j, n])
  kb8    [128,2,4096] fp8   kb8[p,j,m]     = q8(64*(A x)[p+128j, m])
  v8     [128,2,4096] fp8   v8[p,j,g*256+o]= q8(V'[(2g+j)*128+p, o])
  St (DoubleRow): st[m,n] = sum_c kb8[c,m] x8v[c%4][c,n];  Pt = exp(st/(1024 lam))
  OT = V'^T Pt accumulated over 16 key groups; rowsum via (1, g/16) matmul.

Processing in nt-pairs (1024 queries): each stationary operand serves two
consecutive matmuls (amortizes LDWEIGHTS) and each exp covers N=1024.
"""

import sys

sys.path.insert(0, "/opt/trn_rl_repo")

import numpy as np
import concourse.bacc as bacc
import concourse.mybir as mybir
import concourse.tile as tile
from concourse.bass_utils import run_bass_kernel_spmd

F32 = mybir.dt.float32
F32R = mybir.dt.float32r
F8 = mybir.dt.float8e4
AF = mybir.ActivationFunctionType
DR = mybir.MatmulPerfMode.DoubleRow
MULT = mybir.AluOpType.mult
ADD = mybir.AluOpType.add

B = 8
C = 256
NPIX = 4096
NV = 4  # q-side quantization versions
LAM = [2.0 ** (k / (4.0 * NV)) for k in range(NV)]
EXPSCALE = 1.0 / (16.0 * 64.0)  # kb carries 64x

_CACHE = {}


def _build():
    nc = bacc.Bacc("TRN2", num_swdge_queues=4)
    x_d = nc.declare_dram_parameter("x", [C, NPIX], F32, isOutput=False)
    at_d = nc.declare_dram_parameter("at64", [C, C], F32, isOutput=False)
    wvu_d = nc.declare_dram_parameter("wvu", [C, 272], F32, isOutput=False)
    bv_d = nc.declare_dram_parameter("bv272", [1, 272], F32, isOutput=False)
    out_d = nc.declare_dram_parameter("out", [C, NPIX], F32, isOutput=True)

    with tile.TileContext(nc) as tc:
        with (
            tc.tile_pool(name="big", bufs=1) as big,
            tc.tile_pool(name="small", bufs=2) as small,
            tc.tile_pool(name="ptp", bufs=26) as ptp,
            tc.tile_pool(name="outp", bufs=4) as outp,
            tc.tile_pool(name="rbp", bufs=2) as rbp,
            tc.tile_pool(name="psSt", bufs=2, space="PSUM") as psSt,
            tc.tile_pool(name="psOT", bufs=2, space="PSUM") as psOT,
            tc.tile_pool(name="psR", bufs=2, space="PSUM") as psR,
        ):
            # ---------------- static tiles ----------------
            x_f32 = [big.tile([128, NPIX], F32R, name=f"x_f32_{i}") for i in range(2)]
            x8v = [big.tile([128, 2, NPIX], F8, name=f"x8v{k}") for k in range(NV)]
            kb8 = big.tile([128, 2, NPIX], F8, name="kb8")
            v8 = big.tile([128, 2, NPIX], F8, name="v8")
            at_f = big.tile([128, 2, C], F32R, name="at_f")
            wvu_f = big.tile([128, 2, 272], F32R, name="wvu_f")
            bv_sb = big.tile([1, 272], F32R, name="bv_sb")
            ones1p = big.tile([1, 128], F32R, name="ones1p")
            ones2 = big.tile([2, 16], F32R, name="ones2")
            w8 = big.tile([128, 2, 256], F8, name="w8")  # (1, g/16) per group
            gexp = big.tile([128, 32], F32, name="gexp")
            rinv_f = big.tile([1, 1024], F32, name="rinv_f")
            rinv_r = big.tile([1, 1024], F32R, name="rinv_r")
            rs_sb = big.tile([2, 1024], F32R, name="rs_sb")

            # ---------------- warmup (HAM clock + ACT table) ----------------
            warm_f = small.tile([128, 256], F32, name="warm_f", tag="warm_f")
            nc.vector.memset(warm_f, 1.0)
            warm_r = small.tile([128, 256], F32R, name="warm_r", tag="warm_r")
            nc.vector.tensor_copy(warm_r, warm_f)
            # f32r/fp8 constants come from f32 memsets via copy (DVE memset
            # of non-f32 dtypes fails the ISA checker)
            nc.vector.tensor_copy(ones1p, warm_f[0:1, 0:128])
            nc.vector.tensor_copy(ones2, warm_f[0:2, 0:16])
            for j in range(2):
                nc.vector.tensor_copy(w8[:, j, :], warm_f[:, 0:256])
            exp_warm = small.tile([1, 16], F32, name="exp_warm", tag="exp_warm")
            nc.scalar.activation(exp_warm, warm_f[0:1, 0:16], AF.Exp, scale=0.1)
            ones_col = big.tile([128, 1], F32R, name="ones_col")
            nc.vector.tensor_copy(ones_col, warm_f[:, 0:1])
            warm_ps = psR.tile([1, 256], F32, name="warm_ps", tag="psR")
            for _ in range(48):
                nc.tensor.matmul(
                    warm_ps, ones_col, warm_r, start=True, stop=True,
                    skip_group_check=True,
                )

            # ---------------- loads (need-ordered) ----------------
            for i in range(2):
                nc.gpsimd.dma_start(
                    out=at_f[:, i, :], in_=at_d[i * 128 : (i + 1) * 128, :]
                )
            for i in range(2):
                nc.gpsimd.dma_start(
                    out=wvu_f[:, i, :], in_=wvu_d[i * 128 : (i + 1) * 128, :]
                )
            nc.gpsimd.dma_start(out=bv_sb, in_=bv_d[0:1, 0:272])

            for j in range(8):
                lo, hi = j * 512, (j + 1) * 512
                for i in range(2):
                    nc.gpsimd.dma_start(
                        out=x_f32[i][:, lo:hi], in_=x_d[i * 128 : (i + 1) * 128, lo:hi]
                    )

            # ---------------- projections (per x 1024-slice) ----------------
            for jp in range(4):
                nlo = jp * 1024
                if jp == 1:
                    # q-side fp8 versions for nt-pair 0 (rest are emitted
                    # lazily inside the attention loop)
                    for k in range(NV):
                        for i in range(2):
                            nc.vector.tensor_scalar(
                                x8v[k][:, i, 0:1024], x_f32[i][:, 0:1024],
                                LAM[k], None, MULT,
                            )
                # keep the PE busy (and HAM warm) while DMA streams in
                warm_jp = psR.tile([1, 256], F32, name="warm_jp", tag="psR")
                for _ in range(12):
                    nc.tensor.matmul(
                        warm_jp, ones_col, warm_r, start=True, stop=True,
                        skip_group_check=True,
                    )
                # Kb = 64 * A x   (c_out halves)
                for oh in range(2):
                    psk = psSt.tile([128, 1024], F32, name="psk", tag="st")
                    for h in range(2):
                        for i in range(2):
                            nc.tensor.matmul(
                                psk[:, h * 512 : (h + 1) * 512],
                                at_f[:, i, oh * 128 : (oh + 1) * 128],
                                x_f32[i][:, nlo + h * 512 : nlo + (h + 1) * 512],
                                start=(i == 0), stop=(i == 1),
                            )
                    nc.scalar.activation(kb8[:, oh, nlo : nlo + 1024], psk, AF.Copy)
                # V' chunks (4 chunks of 128 pixels per 512-slice => 8 per pair)
                for m in range(jp * 8, (jp + 1) * 8):
                    if m % 2 == 0:
                        psv = psOT.tile([128, 272], F32, name="psv", tag="ot")
                    else:
                        psv = psR.tile([128, 272], F32, name="psvr", tag="psR")
                    for i in range(2):
                        nc.tensor.matmul(
                            psv,
                            x_f32[i][:, m * 128 : (m + 1) * 128],
                            wvu_f[:, i, 0:272],
                            start=(i == 0), stop=False,
                        )
                    nc.tensor.matmul(psv, ones1p, bv_sb, start=False, stop=True)
                    # gexp1 = relu(1 + g/16) = 1 + g/16 ; psv[:,256] holds g
                    nc.scalar.activation(
                        gexp[:, m : m + 1], psv[:, 256:257], AF.Relu,
                        bias=1.0, scale=1.0 / 16.0,
                    )
                    g2 = m // 2
                    nc.vector.tensor_scalar(
                        v8[:, m % 2, g2 * 256 : g2 * 256 + 256],
                        psv[:, 0:256],
                        gexp[:, m : m + 1], None, MULT,
                    )
            # w8 second columns (g/16 = gexp1 - 1), batched strided writes
            for j in range(2):
                nc.vector.tensor_scalar(
                    w8[:, j, 1 : 1 + 15 * 16 + 1 : 16],
                    gexp[:, j : j + 31 : 2],
                    1.0, -1.0, MULT, ADD,
                )

            # ---------------- attention, nt-pairs of 1024 queries ----------------
            LAG = 2
            PF = 5
            pts = {}

            def emit_st_exp(t, g):
                ne, no = 2 * t * 512, (2 * t + 1) * 512
                pt_g = ptp.tile([128, 2, 1024], F8, name="pt")
                for jj in range(2):
                    c = 2 * g + jj
                    k = c % NV
                    st = psSt.tile([128, 1024], F32, name="st", tag="st")
                    for h, qlo in enumerate((ne, no)):
                        nc.tensor.matmul(
                            st[:, h * 512 : (h + 1) * 512],
                            kb8[:, :, c * 128 : (c + 1) * 128],
                            x8v[k][:, :, qlo : qlo + 512],
                            start=True, stop=True, perf_mode=DR,
                        )
                    nc.scalar.activation(
                        pt_g[:, jj, :], st, AF.Exp, scale=EXPSCALE / LAM[k]
                    )
                pts[(t, g)] = pt_g

            for t in range(4):
                rs_ps = [
                    psR.tile([2, 512], F32, name=f"rs_ps{h}", tag="psR")
                    for h in range(2)
                ]
                ot0 = [
                    psOT.tile([128, 512], F32, name=f"ot0_{h}", tag="ot")
                    for h in range(2)
                ]
                if t + 1 < 4:
                    # q-side fp8 versions for the NEXT nt-pair (hides the
                    # casts under this pair's attention)
                    qlo2 = (t + 1) * 1024
                    for k in range(NV):
                        for i in range(2):
                            nc.vector.tensor_scalar(
                                x8v[k][:, i, qlo2 : qlo2 + 1024],
                                x_f32[i][:, qlo2 : qlo2 + 1024],
                                LAM[k], None, MULT,
                            )

                def rs_ot0(g):
                    st_, sp_ = (g == 0), (g == 15)
                    pt_g = pts[(t, g)]
                    for h in range(2):
                        nc.tensor.matmul(
                            rs_ps[h],
                            w8[:, :, g * 16 : g * 16 + 2],
                            pt_g[:, :, h * 512 : (h + 1) * 512],
                            start=st_, stop=sp_, perf_mode=DR,
                        )
                    for h in range(2):
                        nc.tensor.matmul(
                            ot0[h], v8[:, :, g * 256 : g * 256 + 128],
                            pt_g[:, :, h * 512 : (h + 1) * 512],
                            start=st_, stop=sp_, perf_mode=DR,
                        )

                for g in range(16):
                    if (t, g) not in pts:
                        emit_st_exp(t, g)
                    if g >= LAG:
                        rs_ot0(g - LAG)
                for g in range(16 - LAG, 16):
                    rs_ot0(g)

                # den = rs + rsg (fold rows via ones-matmul), reciprocal,
                # broadcast across partitions (ones-matmul again)
                for h in range(2):
                    nc.vector.tensor_copy(
                        rs_sb[:, h * 512 : (h + 1) * 512], rs_ps[h]
                    )
                den_ps = []
                for h in range(2):
                    dp = psR.tile([1, 512], F32, name=f"den_ps{h}", tag="psR")
                    nc.tensor.matmul(
                        dp, ones2[:, 0:1], rs_sb[:, h * 512 : (h + 1) * 512],
                        start=True, stop=True,
                    )
                    den_ps.append(dp)
                rb_sb = []
                for h in range(2):
                    sl = slice(h * 512, (h + 1) * 512)
                    nc.vector.reciprocal_approx_fast(rinv_f[0:1, sl], den_ps[h])
                    nc.vector.tensor_copy(rinv_r[0:1, sl], rinv_f[0:1, sl])
                for h in range(2):
                    sl = slice(h * 512, (h + 1) * 512)
                    rb_ps = psR.tile([128, 512], F32, name="rb_ps", tag="psR")
                    nc.tensor.matmul(
                        rb_ps, ones1p, rinv_r[0:1, sl],
                        start=True, stop=True,
                    )
                    rb = rbp.tile([128, 512], F32, name="rb")
                    nc.vector.tensor_copy(rb, rb_ps)
                    rb_sb.append(rb)

                for h in range(2):
                    osb = outp.tile([128, 512], F32, name="osb", tag="osb")
                    nc.vector.tensor_mul(osb, ot0[h], rb_sb[h])
                    nc.sync.dma_start(
                        out=out_d[0:128, (2 * t + h) * 512 : (2 * t + h + 1) * 512],
                        in_=osb,
                    )

                # pass 2: second half of output channels
                ot1 = [
                    psOT.tile([128, 512], F32, name=f"ot1_{h}", tag="ot")
                    for h in range(2)
                ]
                npf = 0
                for g in range(16):
                    st_, sp_ = (g == 0), (g == 15)
                    pt_g = pts.pop((t, g))
                    for h in range(2):
                        nc.tensor.matmul(
                            ot1[h], v8[:, :, g * 256 + 128 : (g + 1) * 256],
                            pt_g[:, :, h * 512 : (h + 1) * 512],
                            start=st_, stop=sp_, perf_mode=DR,
                        )
                    # prefetch next pair's score/exp stream into this
                    # otherwise ACT-idle window
                    if t + 1 < 4 and npf < PF and g % 3 == 2:
                        emit_st_exp(t + 1, npf)
                        npf += 1
                for h in range(2):
                    osb = outp.tile([128, 512], F32, name="osb2", tag="osb")
                    nc.vector.tensor_mul(osb, ot1[h], rb_sb[h])
                    nc.sync.dma_start(
                        out=out_d[128:256, (2 * t + h) * 512 : (2 * t + h + 1) * 512],
                        in_=osb,
                    )

    nc.compile()
    return nc


def _get_nc():
    if "nc" not in _CACHE:
        _CACHE["nc"] = _build()
    return _CACHE["nc"]


def _host_prep(x, wq, wk, wv, bq, bk, bv):
    x = np.asarray(x, dtype=np.float32)
    wq = np.asarray(wq, dtype=np.float32)
    wk = np.asarray(wk, dtype=np.float32)
    wv = np.asarray(wv, dtype=np.float32)
    bq = np.asarray(bq, dtype=np.float32)
    bv = np.asarray(bv, dtype=np.float32)

    at64 = np.ascontiguousarray(64.0 * (wk.T @ wq))  # [c_in, c_out] = 64*A^T
    wvu = np.zeros((C, 272), dtype=np.float32)
    wvu[:, 0:C] = wv.T
    wvu[:, 256] = wk.T @ bq
    bv272 = np.zeros((1, 272), dtype=np.float32)
    bv272[0, 0:C] = bv
    shared = {
        "at64": at64,
        "wvu": np.ascontiguousarray(wvu),
        "bv272": bv272,
    }
    return [
        {"x": np.ascontiguousarray(x[b].reshape(C, NPIX)), **shared}
        for b in range(B)
    ]


def kernel(x, wq, wk, wv, bq, bk, bv):
    nc = _get_nc()
    in_maps = _host_prep(x, wq, wk, wv, bq, bk, bv)
    res = run_bass_kernel_spmd(nc, in_maps, core_ids=list(range(B)))
    out = np.stack([res.results[b]["out"] for b in range(B)])
    return out.reshape(B, C, 64, 64)
